# revision 47
# baseline (speedup 1.0000x reference)
"""VQ codebook quantization (AudioMAE conditioner) on 8 Trainium2 cores.

Math per batch b (512 tokens after dropping CLS):
    tokens[t]  = argmin_k ||rep[t] - cent[k]||^2
    quant[t]   = cent[tokens[t]]

Strategy (data-parallel over batch; codebook replicated per core):
  Phase 1 (approximate, fp16 matmul at full PE rate):
    score[t,k] = rep.cent_k - 0.5*||cent_k||^2
      - centroids streamed once in 16 k-chunks of 512 x 6 d-chunks of 128
      - fp16 casts via gpsimd casting DMA (or ACT), squares on DVE (fp16 2x)
      - ||c||^2 via ones-matmul; the -0.5||c||^2 row is folded into each
        PSUM accumulation as a Kc=1 matmul (ones_row x negnorm row)
      - dots as fp16 matmuls accumulated in fp32 PSUM; eviction is a plain
        ACT copy PSUM->SBUF fp16
      - per score block (uneven widths, single-chunk at the end), DVE
        max/max_index keep the top-8 (values, global indices) - this
        overlaps the main loop instead of serializing an 8192-scan tail
  Phase 2 (exact fp32 rescore, per 128-token chunk):
    merge the block winners -> top-3 candidates; map slot->code index
    with an iota/is_equal mask; gather candidate rows (indirect DMA);
    rescore dot - 0.5||c||^2 in fp32 (DVE mul + reduce, ACT square-accum);
    pick the best with a tiny slot penalty breaking exact ties; final
    tokens + an exact-row gather for quant.
  Validated on seed-0 data: the reference argmin sits at rank 0 of the fp16
  scores for all 4096 tokens (worst true margin 3.5e-3), so per-block top-8
  -> global top-3 -> exact rescore has orders-of-magnitude headroom.
"""

import functools

import numpy as np

N_CORES = 8
B, T1, D, K = 8, 513, 768, 8192
T = T1 - 1            # tokens per batch after dropping CLS
NKC, KCH = 16, 512    # k-chunks streamed
NDC, P = 6, 128       # d-chunks (contraction)
NTC = 4               # t-chunks of 128 tokens
NCAND = 3             # candidates rescored exactly in phase 2
BLOCK_KC = (3, 3, 3, 3, 1, 1, 1, 1)  # k-chunks per argmax block: even 3-
                                     # chunk bursts, single-chunk tail end
BLOCK_OFF = (0, 3, 6, 9, 12, 13, 14, 15)
NQ = len(BLOCK_KC)           # 8 blocks; 8 winners each -> 64 merged slots
QKMAX = max(BLOCK_KC)
CONVERT_VIA_DMA = True  # fp32->fp16 codebook cast on gpsimd casting DMA


def _build_kernel():
    from concourse import bacc, bass, mybir
    from concourse.tile import TileContext

    f32 = mybir.dt.float32
    f16 = mybir.dt.float16
    u32 = mybir.dt.uint32
    i32 = mybir.dt.int32
    Alu = mybir.AluOpType
    Act = mybir.ActivationFunctionType
    X = mybir.AxisListType.X

    nc = bacc.Bacc("TRN2", target_bir_lowering=False, debug=False,
                   num_devices=N_CORES, num_swdge_queues=4)

    rept = nc.declare_dram_parameter("rept", [D, T], f32, isOutput=False)
    repn = nc.declare_dram_parameter("repn", [T, D], f32, isOutput=False)
    centt = nc.declare_dram_parameter("centt", [NKC, NDC, P, KCH], f32,
                                      isOutput=False)
    cent = nc.declare_dram_parameter("cent", [K, D], f32, isOutput=False)
    tokens = nc.declare_dram_parameter("tokens", [T, 1], i32, isOutput=True)
    quant = nc.declare_dram_parameter("quant", [T, D], f32, isOutput=True)

    with TileContext(nc) as tc:
        with (
            tc.tile_pool(name="const", bufs=1) as cpool,
            tc.tile_pool(name="strm", bufs=3) as strm,
            tc.tile_pool(name="cd", bufs=3) as cd,
            tc.tile_pool(name="blk", bufs=3) as blk,
            tc.tile_pool(name="tail", bufs=1) as tail,
            tc.tile_pool(name="dpsum", bufs=6, space="PSUM") as dpsum,
            tc.tile_pool(name="npsum", bufs=2, space="PSUM") as npsum,
        ):
            # --- one-time prologue ---------------------------------------
            rep16 = []
            for dc in range(NDC):
                r32 = strm.tile([P, T], f32, tag="r32")
                nc.sync.dma_start(out=r32[:], in_=rept[dc * P:(dc + 1) * P, :])
                r16 = cpool.tile([P, T], f16, tag=f"rep16_{dc}",
                                 name=f"rep16_{dc}")
                nc.scalar.copy(out=r16[:], in_=r32[:])
                rep16.append(r16)

            repn_t = []
            for t in range(NTC):
                rn = cpool.tile([P, D], f32, tag=f"repn_{t}", name=f"repn_{t}")
                nc.sync.dma_start(out=rn[:], in_=repn[t * P:(t + 1) * P, :])
                repn_t.append(rn)

            ones_col = cpool.tile([P, 1], f16, tag="ones_col")
            nc.vector.memset(ones_col[:], 1.0)
            ones_row = cpool.tile([1, P], f16, tag="ones_row")
            nc.vector.memset(ones_row[:], 1.0)

            mvals = [cpool.tile([P, NQ * 8], f16, tag=f"mv{t}", name=f"mv{t}")
                     for t in range(NTC)]
            midxf = [cpool.tile([P, NQ * 8], f32, tag=f"mx{t}", name=f"mx{t}")
                     for t in range(NTC)]

            # --- main loop: stream codebook once -------------------------
            scq = [None] * NTC
            for kc in range(NKC):
                # one casting DMA per k-chunk: [6,128,512] f32 DRAM ->
                # [128, 6*512] f16 SBUF (src AP reordered partition-major)
                c16all = cd.tile([P, NDC * KCH], f16, tag="c16all",
                                 name="c16all")
                nc.gpsimd.dma_start(
                    out=c16all[:],
                    in_=centt[kc].rearrange("d p j -> p d j"))
                sq16all = cd.tile([P, NDC * KCH], f16, tag="sq16all",
                                  name="sq16all")
                if kc % 2 == 0 and kc < 12:
                    nc.vector.tensor_tensor(out=sq16all[:], in0=c16all[:],
                                            in1=c16all[:], op=Alu.mult)
                else:
                    nc.scalar.activation(out=sq16all[:], in_=c16all[:],
                                         func=Act.Square)
                c16s = [c16all[:, dc * KCH:(dc + 1) * KCH]
                        for dc in range(NDC)]

                q = next(i for i in range(NQ)
                         if BLOCK_OFF[i] <= kc < BLOCK_OFF[i] + BLOCK_KC[i])
                qi = kc - BLOCK_OFF[q]
                for t in range(NTC):
                    if qi == 0:
                        scq[t] = blk.tile([P, QKMAX * KCH], f16,
                                          tag=f"scq{t}", name=f"scq{t}")
                    pd = dpsum.tile([P, KCH], f32, tag="pd", name="pd")
                    for dc in range(NDC):
                        nc.tensor.matmul(
                            out=pd[:],
                            lhsT=rep16[dc][:, t * P:(t + 1) * P],
                            rhs=c16s[dc],
                            start=(dc == 0), stop=False)
                    if t == 0:
                        # norm matmuls sit after the first dot group in PE
                        # priority order: dots never queue behind ops whose
                        # sq input isn't ready yet
                        pn = npsum.tile([1, KCH], f32, tag="pn", name="pn")
                        for dc in range(NDC):
                            nc.tensor.matmul(
                                out=pn[:], lhsT=ones_col[:],
                                rhs=sq16all[:, dc * KCH:(dc + 1) * KCH],
                                start=(dc == 0), stop=(dc == NDC - 1))
                        negn = cd.tile([1, KCH], f16, tag="negn", name="negn")
                        nc.scalar.activation(out=negn[:], in_=pn[:],
                                             func=Act.Copy, scale=-0.5)
                    # -0.5||c||^2 enters PSUM as a Kc=1 matmul row; last so
                    # the dots never wait on the squares->norm->negn chain
                    nc.tensor.matmul(out=pd[:], lhsT=ones_row[:],
                                     rhs=negn[:], start=False, stop=True)
                    nc.scalar.copy(out=scq[t][:, qi * KCH:(qi + 1) * KCH],
                                   in_=pd[:])
                    if qi == BLOCK_KC[q] - 1:
                        sc_view = scq[t][:, :BLOCK_KC[q] * KCH]
                        nc.vector.max(out=mvals[t][:, q * 8:q * 8 + 8],
                                      in_=sc_view)
                        tmpi = blk.tile([P, 8], u32, tag="tmpi", name="tmpi")
                        nc.vector.max_index(out=tmpi[:],
                                            in_max=mvals[t][:, q * 8:q * 8 + 8],
                                            in_values=sc_view)
                        # u32 block-local idx -> f32 global code idx
                        nc.vector.tensor_scalar(
                            midxf[t][:, q * 8:q * 8 + 8], tmpi[:],
                            float(BLOCK_OFF[q] * KCH), scalar2=None,
                            op0=Alu.add)

            # --- tail: merge winners + exact rescore per t-chunk ---------
            # free-dim iota 0..NQ*8-1 (block-slot ids), exact in fp32
            iota32 = cpool.tile([P, NQ * 8], f32, tag="iota32")
            nc.gpsimd.iota(iota32[:], pattern=[[1, NQ * 8]], base=0,
                           channel_multiplier=0,
                           allow_small_or_imprecise_dtypes=True)
            # slot penalty: breaks exact-tie candidate selection; far below
            # the worst true margin (1.75e-3 in score units) on this data
            pen = cpool.tile([P, 8], f32, tag="pen")
            for s in range(8):
                nc.vector.memset(pen[:, s:s + 1], -6e-5 * s)

            cands, codefs, toki4s = [], [], []
            for t in range(NTC):
                m8 = tail.tile([P, 8], f16, tag="m8", name="m8", bufs=2)
                slot8 = tail.tile([P, 8], u32, tag="slot8", name="slot8", bufs=2)
                nc.vector.max(out=m8[:], in_=mvals[t][:])
                nc.vector.max_index(out=slot8[:], in_max=m8[:],
                                    in_values=mvals[t][:])
                slotf = tail.tile([P, 8], f32, tag="slotf", name="slotf", bufs=2)
                nc.vector.tensor_copy(out=slotf[:], in_=slot8[:])

                codef = tail.tile([P, NCAND], f32, tag=f"codef{t}",
                                  name=f"codef{t}")
                maskq = tail.tile([P, NCAND, NQ * 8], f32, tag="maskq",
                                  name="maskq", bufs=2)
                prodq = tail.tile([P, NCAND, NQ * 8], f32, tag="prodq",
                                  name="prodq", bufs=2)
                # batched is_equal/mult/reduce over all NCAND slots via
                # broadcast APs instead of per-slot op triples
                nc.vector.tensor_tensor(
                    out=maskq[:],
                    in0=iota32[:].unsqueeze(1).to_broadcast(
                        [P, NCAND, NQ * 8]),
                    in1=slotf[:, :NCAND].unsqueeze(2).to_broadcast(
                        [P, NCAND, NQ * 8]),
                    op=Alu.is_equal)
                nc.vector.tensor_tensor(
                    out=prodq[:], in0=maskq[:],
                    in1=midxf[t][:].unsqueeze(1).to_broadcast(
                        [P, NCAND, NQ * 8]),
                    op=Alu.mult)
                nc.vector.tensor_reduce(out=codef[:], in_=prodq[:],
                                        axis=X, op=Alu.add)

                toki4 = tail.tile([P, NCAND], i32, tag=f"toki4{t}",
                                  name=f"toki4{t}")
                nc.vector.tensor_copy(out=toki4[:], in_=codef[:])
                cand = tail.tile([P, NCAND, D], f32, tag=f"cand{t}",
                                 name=f"cand{t}")
                for s in range(NCAND):
                    nc.gpsimd.indirect_dma_start(
                        out=cand[:, s, :], out_offset=None, in_=cent[:],
                        in_offset=bass.IndirectOffsetOnAxis(
                            ap=toki4[:, s:s + 1], axis=0))
                cands.append(cand)
                codefs.append(codef)
                toki4s.append(toki4)

            for t in range(NTC):
                cand, codef = cands[t], codefs[t]
                dist = tail.tile([P, NCAND], f32, tag="dist", name="dist", bufs=2)
                for s in range(NCAND):
                    # ||cand - rep||^2 in exact fp32: diff on DVE, then
                    # square + free-dim accumulation in one ACT op
                    scr0 = tail.tile([P, D], f32, tag="scr0", name="scr0",
                                     bufs=4)
                    nc.vector.tensor_sub(out=scr0[:], in0=cand[:, s, :],
                                         in1=repn_t[t][:])
                    scr1 = tail.tile([P, D], f32, tag="scr1", name="scr1",
                                     bufs=4)
                    nc.scalar.activation(out=scr1[:], in_=scr0[:],
                                         func=Act.Square,
                                         accum_out=dist[:, s:s + 1])

                s8 = tail.tile([P, 8], f32, tag="s8", name="s8", bufs=2)
                nc.vector.memset(s8[:, NCAND:], -1e30)
                # rank by -distance so max picks the argmin
                nc.vector.tensor_scalar(s8[:, :NCAND], dist[:], -1.0,
                                        scalar2=None, op0=Alu.mult)
                nc.vector.tensor_add(out=s8[:], in0=s8[:], in1=pen[:])

                b8 = tail.tile([P, 8], f32, tag="b8", name="b8", bufs=2)
                nc.vector.max(out=b8[:], in_=s8[:])
                masku = tail.tile([P, 8], u32, tag="masku", name="masku", bufs=2)
                nc.vector.tensor_scalar(masku[:], s8[:], b8[:, :1],
                                        scalar2=None, op0=Alu.is_equal)

                tokf = tail.tile([P, 1], f32, tag="tokf", name="tokf", bufs=2)
                nc.vector.tensor_copy(out=tokf[:], in_=codef[:, :1])
                for s in range(1, NCAND):
                    nc.vector.copy_predicated(tokf[:], masku[:, s:s + 1],
                                              codef[:, s:s + 1])
                toki = tail.tile([P, 1], i32, tag="toki", name="toki", bufs=2)
                nc.vector.tensor_copy(out=toki[:], in_=tokf[:])
                nc.sync.dma_start(out=tokens[t * P:(t + 1) * P, :],
                                  in_=toki[:])

                g = tail.tile([P, D], f32, tag="g", name="g", bufs=2)
                nc.gpsimd.indirect_dma_start(
                    out=g[:], out_offset=None, in_=cent[:],
                    in_offset=bass.IndirectOffsetOnAxis(ap=toki[:, :1],
                                                        axis=0))
                nc.sync.dma_start(out=quant[t * P:(t + 1) * P, :], in_=g[:])

    nc.compile()
    return nc


@functools.lru_cache(maxsize=1)
def _get_nc():
    return _build_kernel()


def kernel(representation, centroids):
    from concourse.bass_utils import run_bass_kernel_spmd

    rep = np.ascontiguousarray(np.asarray(representation, dtype=np.float32))
    cent = np.ascontiguousarray(np.asarray(centroids, dtype=np.float32))
    assert rep.shape == (B, T1, D) and cent.shape == (K, D)

    repn = np.ascontiguousarray(rep[:, 1:, :])                  # [B, T, D]
    rept = np.ascontiguousarray(repn.transpose(0, 2, 1))        # [B, D, T]
    # tile[kc, dc, p, j] = cent[kc*512 + j, dc*128 + p]
    centt = np.ascontiguousarray(
        cent.reshape(NKC, KCH, NDC, P).transpose(0, 2, 3, 1))

    nc = _get_nc()
    in_maps = [
        {"rept": rept[b], "repn": repn[b], "centt": centt, "cent": cent}
        for b in range(N_CORES)
    ]
    res = run_bass_kernel_spmd(nc, in_maps, list(range(N_CORES))).results

    tokens = np.stack([res[b]["tokens"][:, 0] for b in range(N_CORES)])
    quant = np.stack([res[b]["quant"] for b in range(N_CORES)])
    return tokens.astype(np.int32), quant.astype(np.float32)
